# revision 3
# baseline (speedup 1.0000x reference)
"""Trainium2 Bass kernel for CompressDCT encoder (per-8x8-block 2D DCT,
quantize, round-nearest-even, clamp).

Approach
--------
For an H x W = 128 x 128 image, the per-8x8-block 2D DCT equals
    Y = K @ img @ K^T,   K = kron(I_16, D)  (128x128 block-diagonal)
which maps perfectly onto the 128x128 systolic array.

Per image:
  pass1: matmul(lhsT=img[h,w], rhs=K^T[h,i])  -> T1t[w,i] = (K@img)^T  (PSUM)
  copy1: ScalarE copies T1t PSUM->SBUF
  pass2: matmul(lhsT=K^T[w,l], rhs=T1t[w,(img,i)]) -> Y^T[l,(img,i)]  (PSUM)
  copy2: VectorE tensor_copy f32->int8 == round-to-nearest-even + saturate
         to [-128,127] (exactly matches round + clip for bit=8)
Output is written transposed ([l, img, i]); the host untransposes.

DMAs are batched 8 images per dma_start (input on ScalarE queue, output on
SyncE queue) — cost-model-tuned; halving DMA count took the modeled
per-core time from ~91us to ~69us against a ~58us DMA roofline.

Sharding: pure data-parallel, x[4i:4(i+1)] per core (8 cores).

Numerics: float32r (TF32-like: 8-bit exp, 11-bit mantissa; 1 PE cycle/row
at N>=256) flips ~0.06% of outputs by exactly 1 quantization step vs the
fp32 reference (absmax 1.0).  Full fp32 matmuls (4 cycles/row) reduce that
to ~5 flips in 33.5M (still absmax 1.0 — PE fp32 is not bit-identical to
XLA-CPU at exact-tie .5 boundaries) but cost ~129us vs ~69us modeled.
Both absmax values are identical; USE_F32R picks the fast one.
"""
import os
import sys

import numpy as np

for _p in ("/opt/trn_rl_repo", "/root/.axon_site/_ro/trn_rl_repo"):
    if _p not in sys.path:
        sys.path.append(_p)

# The bass execute path runs through the axon PJRT proxy; make sure a
# JAX_PLATFORMS override (e.g. cpu-pinned reference runners) doesn't hide
# the axon backend, as long as jax hasn't been imported yet.
if os.environ.get("AXON_H4_ENABLED") == "1" and "jax" not in sys.modules:
    if "axon" not in os.environ.get("JAX_PLATFORMS", "axon"):
        os.environ["JAX_PLATFORMS"] = "axon"

import concourse.bacc as bacc
import concourse.tile as tile
from concourse import mybir
from concourse.bass_utils import run_bass_kernel_spmd

F32 = mybir.dt.float32
F32R = mybir.dt.float32r
I8 = mybir.dt.int8

N_CORES = 8
IMGS_PER_CORE = 256          # (32/8) * 64 images of 128x128
GROUP = 4                    # images per compute sub-group (PSUM-bank sized)
DMA_GROUPS = 2               # compute sub-groups per DMA batch
BLK = GROUP * DMA_GROUPS     # images per DMA batch
N_BLKS = IMGS_PER_CORE // BLK
MAGIC = 12582912.0           # 1.5 * 2**23, fp32 RNE-to-integer trick

USE_F32R = True
LAST_RESULTS = None


def _round_fp32r(x: np.ndarray) -> np.ndarray:
    """RNE-round fp32 to 11 explicit mantissa bits (fp32r storage)."""
    u = np.ascontiguousarray(x, dtype=np.float32).view(np.uint32)
    lsb = (u >> 12) & 1
    r = (u.astype(np.uint64) + 0x7FF + lsb).astype(np.uint32) & np.uint32(0xFFFFF000)
    return r.view(np.float32)


def _dct_matrix() -> np.ndarray:
    n = np.arange(8)
    k = np.arange(8)[:, None]
    D = np.cos(np.pi * (2 * n + 1) * k / 16.0)
    D *= np.sqrt(2.0 / 8.0)
    D[0] *= 1.0 / np.sqrt(2.0)
    return D.astype(np.float32)


def _q_factors(q_table: np.ndarray):
    """Return (rr, cc, recip_full). recip(1/q) == rr[:,None]*cc[None,:] when
    rank-1 (always true for all-ones); recip_full is not None otherwise."""
    recip = (1.0 / q_table.astype(np.float64))
    if np.allclose(recip, 1.0):
        ones = np.ones(8, dtype=np.float32)
        return ones, ones, None
    u, s, vt = np.linalg.svd(recip)
    approx = s[0] * np.outer(u[:, 0], vt[0])
    if np.allclose(approx, recip, rtol=1e-7, atol=1e-9):
        rr = (u[:, 0] * s[0]).astype(np.float32)
        cc = vt[0].astype(np.float32)
        if rr[0] < 0:
            rr, cc = -rr, -cc
        return rr, cc, None
    return None, None, recip.astype(np.float32)


def _build(use_f32r: bool, bit: int, rr, cc, recip_full):
    """Build + compile the per-core Bass program."""
    mmdt = F32R if use_f32r else F32
    int8_out = bit == 8

    D = _dct_matrix()
    K = np.kron(np.eye(16, dtype=np.float32), D)
    KT = np.ascontiguousarray(K.T)

    if rr is not None:
        RR = np.tile(rr, 16).astype(np.float32)   # [128] per H-freq index i
        CC = np.tile(cc, 16).astype(np.float32)   # [128] per W-freq index l
        KT_A = KT * RR[None, :]                   # pass1 rhs (cols = i)
        KT_B = KT * CC[None, :]                   # pass2 lhsT (cols = l)
    else:
        KT_A = KT
        KT_B = KT
    kmat = np.concatenate([KT_A, KT_A, KT_B], axis=1)  # [128, 384]
    if use_f32r:
        kmat = _round_fp32r(kmat)

    nc = bacc.Bacc("TRN2", target_bir_lowering=False, debug=False)
    x_ap = nc.dram_tensor(
        "x", [N_BLKS, BLK, 128, 128], mmdt, kind="ExternalInput"
    ).ap()
    out_dt = I8 if int8_out else F32
    out_ap = nc.dram_tensor(
        "out", [128, N_BLKS, BLK * 128], out_dt, kind="ExternalOutput"
    ).ap()
    KMAT = nc.inline_tensor(kmat, name="kmat")

    if recip_full is not None:
        # R_t[l, (j,i)] = recip[i%8, l%8], tiled for GROUP images
        l_idx = np.arange(128) % 8
        i_idx = np.arange(128) % 8
        r128 = recip_full[np.ix_(i_idx, l_idx)].T        # [l, i]
        r_t = np.tile(r128, (1, GROUP)).astype(np.float32)
        RT = nc.inline_tensor(np.ascontiguousarray(r_t), name="rt")

    lo = float(-(2.0 ** (bit - 1)))
    hi = float(2.0 ** (bit - 1) - 1)

    with tile.TileContext(nc) as tc:
        with tc.tile_pool(name="const", bufs=1) as cpool, \
             tc.tile_pool(name="inp", bufs=3) as ipool, \
             tc.tile_pool(name="mid", bufs=3) as mpool, \
             tc.tile_pool(name="outp", bufs=3) as opool, \
             tc.tile_pool(name="ps1", bufs=2, space="PSUM") as ps1, \
             tc.tile_pool(name="ps2", bufs=2, space="PSUM") as ps2:

            kt = cpool.tile([128, 384], mmdt, tag="kt")
            nc.sync.dma_start(out=kt[:], in_=KMAT.ap().bitcast(mmdt))
            if recip_full is not None:
                rt = cpool.tile([128, GROUP * 128], F32, tag="rt")
                nc.sync.dma_start(out=rt[:], in_=RT.ap())

            for gb in range(N_BLKS):
                # ---- load BLK images: [h, (j, w)], one DMA (ScalarE q) ----
                xin = ipool.tile([128, BLK * 128], mmdt, tag="xin")
                nc.scalar.dma_start(
                    out=xin[:].rearrange("h (j w) -> h j w", j=BLK),
                    in_=x_ap[gb].rearrange("j h w -> h j w"),
                )
                oc = opool.tile([128, BLK * 128], out_dt, tag="oc")

                for sg in range(DMA_GROUPS):
                    xo = sg * GROUP * 128
                    # ---- pass 1: per image T1t = (K @ img)^T ----
                    if use_f32r:
                        # N=256 (rhs = [KT|KT]) for full-rate f32r; per
                        # PSUM bank: img a -> [0:256] (start), b -> [256:512].
                        # Real halves at [0:128] and [256:384].
                        p1 = ps1.tile([128, GROUP * 256], F32, tag="p1")
                        for j in range(GROUP):
                            nc.tensor.matmul(
                                p1[:, j * 256:(j + 1) * 256],
                                xin[:, xo + j * 128:xo + (j + 1) * 128],
                                kt[:, 0:256],
                                start=(j % 2 == 0),
                                stop=(j % 2 == 1),
                            )
                        p1v = p1[:].rearrange(
                            "w (j d x) -> w j d x", j=GROUP, d=2)[:, :, 0]
                    else:
                        p1 = ps1.tile([128, GROUP * 128], F32, tag="p1")
                        for j in range(GROUP):
                            nc.tensor.matmul(
                                p1[:, j * 128:(j + 1) * 128],
                                xin[:, xo + j * 128:xo + (j + 1) * 128],
                                kt[:, 0:128],
                                start=(j == 0),
                                stop=(j == GROUP - 1),
                            )
                        p1v = p1[:].rearrange("w (j x) -> w j x", j=GROUP)

                    # ---- copy1 (ScalarE): PSUM -> SBUF ----
                    t1t = mpool.tile([128, GROUP * 128], mmdt, tag="t1t")
                    nc.scalar.activation(
                        t1t[:].rearrange("w (j x) -> w j x", j=GROUP),
                        p1v,
                        mybir.ActivationFunctionType.Copy,
                    )

                    # ---- pass 2: Y^T[l, (j,i)] ----
                    p2 = ps2.tile([128, GROUP * 128], F32, tag="p2")
                    nc.tensor.matmul(
                        p2[:], kt[:, 256:384], t1t[:], start=True, stop=True
                    )

                    # ---- quantize/round/clamp ----
                    if recip_full is not None:
                        scaled = mpool.tile([128, GROUP * 128], F32, tag="sc")
                        nc.vector.tensor_tensor(
                            scaled[:], p2[:], rt[:], mybir.AluOpType.mult
                        )
                        src = scaled
                    else:
                        src = p2
                    dst = oc[:, xo:xo + GROUP * 128]
                    if int8_out:
                        nc.vector.tensor_copy(dst, src[:])
                    else:
                        rnd = mpool.tile([128, GROUP * 128], F32, tag="rnd")
                        nc.vector.tensor_scalar(
                            rnd[:], src[:], MAGIC, MAGIC,
                            mybir.AluOpType.add, mybir.AluOpType.subtract,
                        )
                        nc.vector.tensor_scalar(
                            dst, rnd[:], hi, lo,
                            mybir.AluOpType.min, mybir.AluOpType.max,
                        )

                # ---- store BLK images, one DMA (SyncE q) ----
                nc.sync.dma_start(out=out_ap[:, gb], in_=oc[:])

    nc.compile()
    return nc


_CACHE = {}


def _get_program(use_f32r: bool, bit: int, q_key, rr, cc, recip_full):
    key = (use_f32r, bit, q_key)
    if key not in _CACHE:
        _CACHE[key] = _build(use_f32r, bit, rr, cc, recip_full)
    return _CACHE[key]


def kernel(x, q_table, bit):
    x = np.asarray(x, dtype=np.float32)
    q_table = np.asarray(q_table, dtype=np.float32)
    bit = int(bit)
    N, C, H, W = x.shape
    assert (H, W) == (128, 128) and N * C == N_CORES * IMGS_PER_CORE, (
        "kernel hardcoded for (32,64,128,128)"
    )

    rr, cc, recip_full = _q_factors(q_table)
    q_key = q_table.tobytes()
    nc = _get_program(USE_F32R, bit, q_key, rr, cc, recip_full)

    flat = x.reshape(N_CORES, IMGS_PER_CORE, 128, 128)
    in_maps = []
    for i in range(N_CORES):
        shard = flat[i].reshape(N_BLKS, BLK, 128, 128)
        if USE_F32R:
            shard = _round_fp32r(shard)
        in_maps.append({"x": np.ascontiguousarray(shard)})

    res = run_bass_kernel_spmd(nc, in_maps, core_ids=list(range(N_CORES)))
    global LAST_RESULTS
    LAST_RESULTS = res

    parts = []
    for i in range(N_CORES):
        o = res.results[i]["out"].reshape(128, IMGS_PER_CORE, 128)
        # device layout [l, img, i] -> [img, i, l]
        parts.append(np.ascontiguousarray(o.transpose(1, 2, 0)))
    out = np.stack(parts).reshape(N, C, H, W).astype(np.float32)
    return out



# revision 4
# speedup vs baseline: 1.6674x; 1.6674x over previous
"""Trainium2 Bass kernel for CompressDCT encoder (per-8x8-block 2D DCT,
quantize, round-nearest-even, clamp).

Approach: single-pass kron DCT
------------------------------
For an 8x8 block X, the 2D DCT is Y = D X D^T, i.e. in vectorized form
    y_vec = (D (x) D) x_vec,          (x) = Kronecker product, 64x64.
Two 8x8 blocks are packed per 128-partition column, so ONE stationary
matrix  KRON2 = blockdiag(D(x)D, D(x)D)  [128x128] performs the whole
2D DCT for two blocks per moving column in a single matmul pass:

    out[o, f] = sum_p KRON2[o, p] * x[p, f]

with p = 64*parity + (r*8+c)  (input sample index inside the block pair)
and  o = 64*parity + (k*8+l)  (DCT coefficient index).   The host packs
x into this [128, 32768] layout (32768 block-pairs per core) and unpacks
the result; both are cheap numpy reshapes/transposes off-device.

Per core: 64 matmuls (N=512, fp16 -> 1 cycle/row, single stationary for
the whole kernel), then PSUM fp32 -> SBUF int8 saturating copies
(alternating ScalarE/VectorE; int8 conversion is exactly
round-nearest-even + clamp to [-128,127], matching bit=8), then
contiguous-line DMAs (4KB descriptors in, 2KB out).

The quantization table is folded EXACTLY into the stationary matrix
(rows of KRON2 scaled by 1/q[k,l]) - no rank-1 requirement.

Numerics: fp16 inputs + fp16 stationary + fp32 PSUM accumulation flips
~0.02% of outputs by exactly 1 quantization step vs the fp32 reference
(CPU sim: 6594/33.5M, L2 rel 0.0135 vs gate 2e-2; baseline f32r 2-pass
was 5344 flips / 0.0121).

Sharding: pure data-parallel, images [256*i : 256*(i+1)] per core.
"""
import os
import sys

import numpy as np

for _p in ("/opt/trn_rl_repo", "/root/.axon_site/_ro/trn_rl_repo"):
    if _p not in sys.path:
        sys.path.append(_p)

# The bass execute path runs through the axon PJRT proxy; make sure a
# JAX_PLATFORMS override (e.g. cpu-pinned reference runners) doesn't hide
# the axon backend, as long as jax hasn't been imported yet.
if os.environ.get("AXON_H4_ENABLED") == "1" and "jax" not in sys.modules:
    if "axon" not in os.environ.get("JAX_PLATFORMS", "axon"):
        os.environ["JAX_PLATFORMS"] = "axon"

import concourse.bacc as bacc
import concourse.tile as tile
from concourse import mybir
from concourse.bass_utils import run_bass_kernel_spmd

F16 = mybir.dt.float16
F32 = mybir.dt.float32
I8 = mybir.dt.int8

N_CORES = 8
P = 128                      # partitions = 2 blocks x 64 samples
IMGS_PER_CORE = 256          # (32/8) * 64 images of 128x128
COLS = IMGS_PER_CORE * 256 // 2   # block-pairs per core = 32768
DMA_CHUNK = 2048             # columns per input/output DMA
COPY_CHUNK = 1024            # columns per PSUM->SBUF copy
MM_CHUNK = 512               # columns per matmul (one PSUM bank fp32)
MAGIC = 12582912.0           # 1.5 * 2**23, fp32 RNE-to-integer trick

LAST_RESULTS = None


def _dct_matrix() -> np.ndarray:
    n = np.arange(8)
    k = np.arange(8)[:, None]
    D = np.cos(np.pi * (2 * n + 1) * k / 16.0)
    D *= np.sqrt(2.0 / 8.0)
    D[0] *= 1.0 / np.sqrt(2.0)
    return D.astype(np.float64)


def _build(bit: int, q_table: np.ndarray):
    """Build + compile the per-core Bass program."""
    int8_out = bit == 8

    D = _dct_matrix()
    KR = np.kron(D, D)                       # [64 (k,l), 64 (r,c)]
    KR = KR / q_table.astype(np.float64).reshape(64)[:, None]
    KR2 = np.kron(np.eye(2), KR)             # [128, 128] block-diagonal
    kmat = np.ascontiguousarray(KR2.T).astype(np.float16)  # lhsT [p, o]

    nc = bacc.Bacc("TRN2", target_bir_lowering=False, debug=False)
    x_ap = nc.dram_tensor("x", [P, COLS], F16, kind="ExternalInput").ap()
    out_dt = I8 if int8_out else F32
    out_ap = nc.dram_tensor("out", [P, COLS], out_dt, kind="ExternalOutput").ap()
    KMAT = nc.inline_tensor(kmat, name="kmat")

    lo = float(-(2.0 ** (bit - 1)))
    hi = float(2.0 ** (bit - 1) - 1)

    n_dma = COLS // DMA_CHUNK                # 16
    n_copy = DMA_CHUNK // COPY_CHUNK         # 2 per DMA chunk
    n_mm = COPY_CHUNK // MM_CHUNK            # 2 per copy chunk

    with tile.TileContext(nc) as tc:
        with tc.tile_pool(name="const", bufs=1) as cpool, \
             tc.tile_pool(name="inp", bufs=3) as ipool, \
             tc.tile_pool(name="outp", bufs=3) as opool, \
             tc.tile_pool(name="mid", bufs=2) as mpool, \
             tc.tile_pool(name="ps", bufs=3, space="PSUM") as pspool:

            kt = cpool.tile([P, P], F16, tag="kt")
            nc.sync.dma_start(out=kt[:], in_=KMAT.ap())

            ci = 0                           # global copy index
            for c in range(n_dma):
                xin = ipool.tile([P, DMA_CHUNK], F16, tag="xin")
                nc.gpsimd.dma_start(
                    out=xin[:],
                    in_=x_ap[:, c * DMA_CHUNK:(c + 1) * DMA_CHUNK],
                )
                oc = opool.tile([P, DMA_CHUNK], out_dt, tag="oc")

                for k in range(n_copy):
                    ps = pspool.tile([P, COPY_CHUNK], F32, tag="ps")
                    for m in range(n_mm):
                        xo = k * COPY_CHUNK + m * MM_CHUNK
                        nc.tensor.matmul(
                            ps[:, m * MM_CHUNK:(m + 1) * MM_CHUNK],
                            kt[:],
                            xin[:, xo:xo + MM_CHUNK],
                            start=True,
                            stop=True,
                        )
                    dst = oc[:, k * COPY_CHUNK:(k + 1) * COPY_CHUNK]
                    if int8_out:
                        # fp32 -> int8 == RNE + saturate to [-128,127]
                        if ci % 2 == 0:
                            nc.scalar.activation(
                                dst, ps[:], mybir.ActivationFunctionType.Copy
                            )
                        else:
                            nc.vector.tensor_copy(dst, ps[:])
                    else:
                        rnd = mpool.tile([P, COPY_CHUNK], F32, tag="rnd")
                        nc.vector.tensor_scalar(
                            rnd[:], ps[:], MAGIC, MAGIC,
                            mybir.AluOpType.add, mybir.AluOpType.subtract,
                        )
                        nc.vector.tensor_scalar(
                            dst, rnd[:], hi, lo,
                            mybir.AluOpType.min, mybir.AluOpType.max,
                        )
                    ci += 1

                nc.sync.dma_start(
                    out=out_ap[:, c * DMA_CHUNK:(c + 1) * DMA_CHUNK],
                    in_=oc[:],
                )

    nc.compile()
    return nc


_CACHE = {}


def _get_program(bit: int, q_table: np.ndarray):
    key = (bit, q_table.tobytes())
    if key not in _CACHE:
        _CACHE[key] = _build(bit, q_table)
    return _CACHE[key]


def _pack(x: np.ndarray) -> np.ndarray:
    """(N, C, 128, 128) fp32 -> [cores, 128, COLS] fp16 device layout."""
    N, C, H, W = x.shape
    x16 = x.astype(np.float16)
    # -> (cores, img, bh, r, bw, c) -> (cores, img, bh, bw, r, c)
    b = x16.reshape(N_CORES, IMGS_PER_CORE, 16, 8, 16, 8)
    b = b.transpose(0, 1, 2, 4, 3, 5).reshape(N_CORES, -1, 64)  # [cores, B, 64]
    # block index b = f*2 + parity ; partition = parity*64 + (r*8+c)
    b = b.reshape(N_CORES, COLS, 2, 64).transpose(0, 2, 3, 1)   # [cores, 2, 64, COLS]
    return np.ascontiguousarray(b.reshape(N_CORES, P, COLS))


def _unpack(dev: np.ndarray) -> np.ndarray:
    """[cores, 128, COLS] -> (cores*IMGS, 128, 128) float32."""
    d = dev.reshape(N_CORES, 2, 64, COLS).transpose(0, 3, 1, 2)  # [cores, COLS, 2, 64]
    d = d.reshape(N_CORES, IMGS_PER_CORE, 16, 16, 8, 8)          # (img, bh, bw, k, l)
    d = d.transpose(0, 1, 2, 4, 3, 5)                            # (img, bh, k, bw, l)
    return d.reshape(N_CORES * IMGS_PER_CORE, 128, 128).astype(np.float32)


def kernel(x, q_table, bit):
    x = np.asarray(x, dtype=np.float32)
    q_table = np.asarray(q_table, dtype=np.float32)
    bit = int(bit)
    N, C, H, W = x.shape
    assert (H, W) == (128, 128) and N * C == N_CORES * IMGS_PER_CORE, (
        "kernel hardcoded for (32,64,128,128)"
    )

    nc = _get_program(bit, q_table)

    packed = _pack(x)
    in_maps = [{"x": packed[i]} for i in range(N_CORES)]

    res = run_bass_kernel_spmd(nc, in_maps, core_ids=list(range(N_CORES)))
    global LAST_RESULTS
    LAST_RESULTS = res

    dev = np.stack([res.results[i]["out"] for i in range(N_CORES)])
    out = _unpack(dev).reshape(N, C, H, W)
    return out
